# revision 6
# baseline (speedup 1.0000x reference)
"""Trainium2 Bass kernel for nn_ConcatSquashLinearSA.

Math (per sample b, S=1):
    gate = sigmoid(ctx @ Wg.T + bg)          [256]
    bias = ctx @ Wb.T                        [256]
    kv   = ctx @ Wkv.T                       [256]
    E    = outer(kv, kv)                     [256,256]
    A    = softmax_rows(E)
    att  = A / (1e-9 + colsum(A))
    out  = (x @ Wl.T + bl) @ (I + att) * gate + bias

Folded for the device (U = Wl.T, g = gate, cs = colsum(A)):
    Q    = U @ A @ diag(1/cs) + U            [256,256]  (tiny, on-device)
    out^T = diag(g) (Q^T x^T) + diag(g)(bl + (bl@A)/cs) + bias
i.e. the big op is a single matmul against Q with a PER-PARTITION scale
(g[e]) and bias (b_fin[e]) applied while draining PSUM -> fp16 SBUF.

Sharding: data-parallel over batch, 2 samples per core across 8 cores.

Dataflow per core (memory-bound problem -> minimize HBM bytes):
  * x is downcast to fp16 and pre-transposed on the HOST: xT [256, 32768]
    with the contraction dim k on partitions. No on-chip transposes.
  * Weight-stationary main loop: lhsT = Q k-half/e-half [128,128], rhs =
    xT chunk [128,512]; psum tile [128e, 512n]. Scale+bias drain splits
    between Scalar (Identity w/ scale+bias APs) and Vector engines.
  * Output stored transposed (outT [256, 32768] fp16), 4 KiB contiguous
    per partition; host transposes back / upcasts. HBM per core:
    16.8 MB in + 16.8 MB out (~93 us at 360 GB/s = the roofline).
  * Setup latency is hidden: consts packed into 4 DMAs, PE warmed up
    with dummy matmuls, only exp/identity activations (single table),
    all gate/bias vectors computed column-shaped ([128,2], fast DVE),
    and 5 macro-tiles of x prefetch cover the remaining latency.
"""

import numpy as np

B, N, DIN, DOUT, DCTX = 16, 16384, 256, 256, 131
NCORES = 8
SPC = B // NCORES           # samples per core
ROWS = SPC * N              # x rows per core
MACRO = 2048                # n-columns of xT per macro-tile
KC = DCTX + 1               # ctx rows incl. the constant-1 row (b_gate)
USE_F32R = False            # kept for test.py compat; fp16 path ignores it


def build_nc(rows=ROWS, use_f32r=USE_F32R):
    import concourse.bass as bass  # noqa: F401
    import concourse.tile as tile
    from concourse import bacc, mybir
    from contextlib import ExitStack

    f32 = mybir.dt.float32
    f16 = mybir.dt.float16
    AF = mybir.ActivationFunctionType
    AX = mybir.AxisListType
    OP = mybir.AluOpType

    n_macro = rows // MACRO
    mps = rows // SPC // MACRO   # macro-tiles per sample
    NQ = MACRO // 512            # 512-col n-chunks per macro
    KB = KC - 128                # ctx rows in the second (short) pack

    nc = bacc.Bacc()
    xT_d = nc.declare_dram_parameter("xT", [256, rows], f16, isOutput=False)
    # cpack: rows = ctx-k (incl const-1 row); cols = [ctx_s | Wg.T | Wb.T | Wkv.T]
    cpA_d = nc.declare_dram_parameter("cpackA", [128, 770], f32, isOutput=False)
    cpB_d = nc.declare_dram_parameter("cpackB", [KB, 770], f32, isOutput=False)
    # pk2: rows = half of d/k; cols = [W_layer (256) | W_layer.T (256) | bl2 (2)]
    pk2a_d = nc.declare_dram_parameter("pk2a", [128, 514], f32, isOutput=False)
    pk2b_d = nc.declare_dram_parameter("pk2b", [128, 514], f32, isOutput=False)
    outT_d = nc.declare_dram_parameter("outT", [256, rows], f16, isOutput=True)

    with tile.TileContext(nc) as tc, ExitStack() as ctx:
        consts = ctx.enter_context(tc.tile_pool(name="consts", bufs=1))
        spool = ctx.enter_context(tc.tile_pool(name="scratch", bufs=2))
        perm = ctx.enter_context(tc.tile_pool(name="persample", bufs=1))
        pps = ctx.enter_context(tc.tile_pool(name="pps", bufs=4, space="PSUM"))
        pout = ctx.enter_context(tc.tile_pool(name="pout", bufs=4, space="PSUM"))
        xin = ctx.enter_context(tc.tile_pool(name="xin", bufs=5))
        osb = ctx.enter_context(tc.tile_pool(name="osb", bufs=3))

        # ---- constants-by-memset + PE warmup (no DMA dependencies) ----
        onesr = consts.tile([1, 128], f32, name="onesr", tag="onesr")
        nc.gpsimd.memset(onesr, 1.0)
        onesc = consts.tile([128, 1], f32, name="onesc", tag="onesc")
        nc.gpsimd.memset(onesc, 1.0)
        wm = consts.tile([128, 512], f16, name="wm", tag="wm")
        nc.gpsimd.memset(wm, 0.0)
        for w in range(4):
            wp_ = pout.tile([128, 512], f32, name=f"warm{w}", tag="op")
            nc.tensor.matmul(wp_, lhsT=wm[:, 0:128], rhs=wm, start=True,
                             stop=True)

        def cload(name, dram_ap, shape, dt=f32):
            t = consts.tile(shape, dt, name=name, tag=name)
            nc.sync.dma_start(t, dram_ap)
            return t

        cpA = cload("cpA", cpA_d[:, :], [128, 770])
        cpB = cload("cpB", cpB_d[:, :], [KB, 770])
        pk2a = cload("pk2a", pk2a_d[:, :], [128, 514])
        pk2b = cload("pk2b", pk2b_d[:, :], [128, 514])
        bl2 = pk2a[:, 512:514]

        # ================= per-sample setup, stage-interleaved ==========
        S = range(SPC)
        ckv, kv, gp, eg, ga, gateT, bt, btS = {}, {}, {}, {}, {}, {}, {}, {}
        E, expE, rs, rc, A = {}, {}, {}, {}, {}
        csr, rcs, cb, CSi, ct, rcT, qa, wpj = {}, {}, {}, {}, {}, {}, {}, {}
        weff, bfT = {}, {}

        for s in S:   # ctx projections: kv row + gate/bias columns
            ckv[s] = pps.tile([1, 256], f32, name=f"ckv{s}", tag="ps")
            nc.tensor.matmul(ckv[s], lhsT=cpA[:, s:s + 1], rhs=cpA[:, 514:770],
                             start=True, stop=False)
            nc.tensor.matmul(ckv[s], lhsT=cpB[:, s:s + 1], rhs=cpB[:, 514:770],
                             start=False, stop=True)
            gp[s] = pps.tile([128, 2], f32, name=f"gp{s}", tag="ps")
            bt[s] = pps.tile([128, 2], f32, name=f"bt{s}", tag="ps")
            for h in range(2):
                c0 = 2 + 128 * h
                nc.tensor.matmul(gp[s][:, h:h + 1], lhsT=cpA[:, c0:c0 + 128],
                                 rhs=cpA[:, s:s + 1], start=True, stop=False)
                nc.tensor.matmul(gp[s][:, h:h + 1], lhsT=cpB[:, c0:c0 + 128],
                                 rhs=cpB[:, s:s + 1], start=False, stop=True)
                nc.tensor.matmul(bt[s][:, h:h + 1],
                                 lhsT=cpA[:, c0 + 256:c0 + 384],
                                 rhs=cpA[:, s:s + 1], start=True, stop=False)
                nc.tensor.matmul(bt[s][:, h:h + 1],
                                 lhsT=cpB[:, c0 + 256:c0 + 384],
                                 rhs=cpB[:, s:s + 1], start=False, stop=True)
        for s in S:   # move ctx projections off PSUM; gate = 1/(1+e^-x)
            kv[s] = spool.tile([1, 256], f32, name=f"kv{s}", tag="kv")
            nc.vector.tensor_copy(kv[s], ckv[s])
            eg[s] = spool.tile([128, 2], f32, name=f"eg{s}", tag="eg")
            nc.scalar.activation(eg[s], gp[s], AF.Exp, scale=-1.0)
            btS[s] = spool.tile([128, 2], f32, name=f"btS{s}", tag="btS")
            nc.vector.tensor_copy(btS[s], bt[s])
            ga[s] = spool.tile([128, 2], f32, name=f"ga{s}", tag="ga")
            nc.gpsimd.tensor_scalar_add(ga[s], eg[s], 1.0)
            gateT[s] = perm.tile([128, 2], f32, name=f"gateT{s}",
                                 tag=f"gateT{s}")
            nc.vector.reciprocal(gateT[s], ga[s])
        for s in S:   # E = outer(kv, kv); softmax rows (no max-sub needed)
            for i in range(2):
                E[(s, i)] = pps.tile([128, 256], f32, name=f"E{s}{i}", tag="ps")
                nc.tensor.matmul(E[(s, i)],
                                 lhsT=kv[s][0:1, 128 * i:128 * (i + 1)],
                                 rhs=kv[s], start=True, stop=True)
                expE[(s, i)] = spool.tile([128, 256], f32, name=f"expE{s}{i}",
                                          tag="expE")
                nc.scalar.activation(expE[(s, i)], E[(s, i)], AF.Exp)
                rs[(s, i)] = spool.tile([128, 1], f32, name=f"rs{s}{i}", tag="rs")
                nc.vector.reduce_sum(rs[(s, i)], expE[(s, i)], axis=AX.X)
                rc[(s, i)] = spool.tile([128, 1], f32, name=f"rc{s}{i}", tag="rc")
                nc.vector.reciprocal(rc[(s, i)], rs[(s, i)])
                A[(s, i)] = spool.tile([128, 256], f32, name=f"A{s}{i}",
                                       tag=f"A{s}{i}")
                nc.vector.tensor_scalar_mul(A[(s, i)], expE[(s, i)], rc[(s, i)])
        for s in S:   # colsum row -> 1/cs -> broadcast [128,256]
            csr[s] = pps.tile([1, 256], f32, name=f"csr{s}", tag="ps")
            nc.tensor.matmul(csr[s], lhsT=onesc, rhs=A[(s, 0)],
                             start=True, stop=False)
            nc.tensor.matmul(csr[s], lhsT=onesc, rhs=A[(s, 1)],
                             start=False, stop=True)
            rcs[s] = spool.tile([1, 256], f32, name=f"rcs{s}", tag="rcs")
            nc.vector.reciprocal(rcs[s], csr[s])
            cb[s] = pps.tile([128, 256], f32, name=f"cb{s}", tag="ps")
            nc.tensor.matmul(cb[s], lhsT=onesr, rhs=rcs[s], start=True,
                             stop=True)
            CSi[s] = spool.tile([128, 256], f32, name=f"CSi{s}", tag="CSi")
            nc.vector.tensor_copy(CSi[s], cb[s])
        for s in S:   # wp = U @ A ; Q = wp * (1/cs) + U  (fp16)
            for j in range(2):
                wpj[(s, j)] = pps.tile([128, 256], f32, name=f"wp{s}{j}",
                                       tag="ps")
                nc.tensor.matmul(wpj[(s, j)], lhsT=pk2a[:, 128 * j:128 * (j + 1)],
                                 rhs=A[(s, 0)], start=True, stop=False)
                nc.tensor.matmul(wpj[(s, j)], lhsT=pk2b[:, 128 * j:128 * (j + 1)],
                                 rhs=A[(s, 1)], start=False, stop=True)
                qm = spool.tile([128, 256], f32, name=f"qm{s}{j}", tag="qm")
                nc.vector.tensor_mul(qm, wpj[(s, j)], CSi[s])
                weff[(s, j)] = perm.tile([128, 256], f16, name=f"weff{s}{j}",
                                         tag=f"weff{s}{j}")
                U_half = pk2a[:, 256:512] if j == 0 else pk2b[:, 256:512]
                nc.gpsimd.tensor_add(weff[(s, j)], qm, U_half)
        for s in S:   # b_fin columns: g*(bl + (bl@A)/cs) + bias
            ct[s] = pps.tile([128, 2], f32, name=f"ct{s}", tag="ps")
            qa[s] = pps.tile([128, 2], f32, name=f"qa{s}", tag="ps")
            for h in range(2):
                hs = slice(128 * h, 128 * (h + 1))
                nc.tensor.matmul(ct[s][:, h:h + 1], lhsT=A[(s, 0)][:, hs],
                                 rhs=onesc, start=True, stop=False)
                nc.tensor.matmul(ct[s][:, h:h + 1], lhsT=A[(s, 1)][:, hs],
                                 rhs=onesc, start=False, stop=True)
                nc.tensor.matmul(qa[s][:, h:h + 1], lhsT=A[(s, 0)][:, hs],
                                 rhs=bl2[:, 0:1], start=True, stop=False)
                nc.tensor.matmul(qa[s][:, h:h + 1], lhsT=A[(s, 1)][:, hs],
                                 rhs=bl2[:, 1:2], start=False, stop=True)
            rcT[s] = spool.tile([128, 2], f32, name=f"rcT{s}", tag="rcT")
            nc.vector.reciprocal(rcT[s], ct[s])
            f1 = spool.tile([128, 2], f32, name=f"f1{s}", tag="f1")
            nc.vector.tensor_mul(f1, qa[s], rcT[s])
            f2 = spool.tile([128, 2], f32, name=f"f2{s}", tag="f2")
            nc.gpsimd.tensor_add(f2, f1, bl2)
            f3 = spool.tile([128, 2], f32, name=f"f3{s}", tag="f3")
            nc.gpsimd.tensor_mul(f3, f2, gateT[s])
            bfT[s] = perm.tile([128, 2], f32, name=f"bfT{s}", tag=f"bfT{s}")
            nc.vector.tensor_add(bfT[s], f3, btS[s])

        # ================= main loop ====================================
        for t in range(n_macro):
            s = t // mps
            n0 = MACRO * t
            xa = xin.tile([128, MACRO], f16, name="xa", tag="xa")
            nc.sync.dma_start(xa, xT_d[0:128, n0:n0 + MACRO])
            xb = xin.tile([128, MACRO], f16, name="xb", tag="xb")
            nc.sync.dma_start(xb, xT_d[128:256, n0:n0 + MACRO])
            for h in range(2):
                gcol = gateT[s][:, h:h + 1]
                bcol = bfT[s][:, h:h + 1]
                ot = osb.tile([128, MACRO], f16, name=f"ot{h}", tag=f"ot{h}")
                for q in range(NQ):
                    op = pout.tile([128, 512], f32, name="op", tag="op")
                    nc.tensor.matmul(op, lhsT=weff[(s, 0)][:, 128 * h:128 * (h + 1)],
                                     rhs=xa[:, 512 * q:512 * (q + 1)],
                                     start=True, stop=False)
                    nc.tensor.matmul(op, lhsT=weff[(s, 1)][:, 128 * h:128 * (h + 1)],
                                     rhs=xb[:, 512 * q:512 * (q + 1)],
                                     start=False, stop=True)
                    dst = ot[:, 512 * q:512 * (q + 1)]
                    if q % 2 == 0:
                        nc.scalar.activation(dst, op, AF.Identity,
                                             bias=bcol, scale=gcol)
                    else:
                        nc.vector.tensor_scalar(dst, op, gcol, bcol,
                                                op0=OP.mult, op1=OP.add)
                if t == n_macro - 1:
                    for q in range(NQ):   # finer stores to shorten the drain
                        nc.gpsimd.dma_start(
                            outT_d[128 * h:128 * (h + 1),
                                   n0 + 512 * q:n0 + 512 * (q + 1)],
                            ot[:, 512 * q:512 * (q + 1)])
                else:
                    nc.gpsimd.dma_start(
                        outT_d[128 * h:128 * (h + 1), n0:n0 + MACRO], ot)

    nc.finalize()
    return nc


def prep_host_inputs(ctx, x, W_layer, b_layer, W_bias, W_gate, b_gate, W_kv,
                     rows=ROWS):
    """Build the per-core in_maps (host-side sharding + fp16 relayout)."""
    ctx = np.asarray(ctx, np.float32)
    W_layer = np.asarray(W_layer, np.float32)
    b_layer = np.asarray(b_layer, np.float32)
    W_bias = np.asarray(W_bias, np.float32)
    W_gate = np.asarray(W_gate, np.float32)
    b_gate = np.asarray(b_gate, np.float32)
    W_kv = np.asarray(W_kv, np.float32)
    x16 = np.asarray(x).astype(np.float16)

    wcatT = np.zeros((KC, 768), np.float32)
    wcatT[:DCTX, 0:256] = W_gate.T
    wcatT[:DCTX, 256:512] = W_bias.T
    wcatT[:DCTX, 512:768] = W_kv.T
    wcatT[DCTX, 0:256] = b_gate        # paired with the constant-1 ctx row

    bl2 = b_layer.reshape(2, 128).T            # bl2[p, h] = bl[128h+p]
    pk2a = np.concatenate([W_layer[0:128, :], W_layer.T[0:128, :], bl2],
                          axis=1).astype(np.float32)
    pk2b = np.concatenate([W_layer[128:256, :], W_layer.T[128:256, :], bl2],
                          axis=1).astype(np.float32)
    shared = {"pk2a": np.ascontiguousarray(pk2a),
              "pk2b": np.ascontiguousarray(pk2b)}
    in_maps = []
    for c in range(NCORES):
        ctxc = np.zeros((KC, SPC), np.float32)
        for k in range(SPC):
            ctxc[:DCTX, k] = ctx[SPC * c + k, 0]
            ctxc[DCTX, k] = 1.0
        cpack = np.concatenate([ctxc, wcatT], axis=1)   # [KC, 770]
        xT = np.ascontiguousarray(
            x16[SPC * c:SPC * (c + 1)].reshape(rows, DIN).T)
        in_maps.append({
            "xT": xT,
            "cpackA": np.ascontiguousarray(cpack[0:128]),
            "cpackB": np.ascontiguousarray(cpack[128:KC]),
            **shared,
        })
    return in_maps


def kernel(ctx, x, W_layer, b_layer, W_bias, W_gate, b_gate, W_kv):
    from concourse.bass_utils import run_bass_kernel_spmd

    nc = build_nc(ROWS)
    in_maps = prep_host_inputs(ctx, x, W_layer, b_layer, W_bias, W_gate,
                               b_gate, W_kv)
    res = run_bass_kernel_spmd(nc, in_maps, core_ids=list(range(NCORES)))
    out = np.empty((B, N, DOUT), np.float32)
    for c in range(NCORES):
        outT = res.results[c]["outT"]           # [256, ROWS] fp16
        out[SPC * c:SPC * (c + 1)] = (
            outT.T.reshape(SPC, N, DOUT).astype(np.float32))
    return out


# revision 9
# speedup vs baseline: 1.0024x; 1.0024x over previous
"""Trainium2 Bass kernel for nn_ConcatSquashLinearSA.

Math (per sample b, S=1):
    gate = sigmoid(ctx @ Wg.T + bg)          [256]
    bias = ctx @ Wb.T                        [256]
    kv   = ctx @ Wkv.T                       [256]
    E    = outer(kv, kv)                     [256,256]
    A    = softmax_rows(E)
    att  = A / (1e-9 + colsum(A))
    out  = (x @ Wl.T + bl) @ (I + att) * gate + bias

Folded for the device (U = Wl.T, g = gate, cs = colsum(A)):
    Q    = U @ A @ diag(1/cs) + U            [256,256]  (tiny, on-device)
    out^T = diag(g) (Q^T x^T) + diag(g)(bl + (bl@A)/cs) + bias
i.e. the big op is a single matmul against Q with a PER-PARTITION scale
(g[e]) and bias (b_fin[e]) applied while draining PSUM -> fp16 SBUF.

Sharding: data-parallel over batch, 2 samples per core across 8 cores.

Dataflow per core (memory-bound problem -> minimize HBM bytes):
  * x is downcast to fp16 and pre-transposed on the HOST: xT [256, 32768]
    with the contraction dim k on partitions. No on-chip transposes.
  * Weight-stationary main loop: lhsT = Q k-half/e-half [128,128], rhs =
    xT chunk [128,512]; psum tile [128e, 512n]. Scale+bias drain splits
    between Scalar (Identity w/ scale+bias APs) and Vector engines.
  * Output stored transposed (outT [256, 32768] fp16), 4 KiB contiguous
    per partition; host transposes back / upcasts. HBM per core:
    16.8 MB in + 16.8 MB out (~93 us at 360 GB/s = the roofline).
  * Setup latency is hidden: consts packed into 4 DMAs, PE warmed up
    with dummy matmuls, only exp/identity activations (single table),
    all gate/bias vectors computed column-shaped ([128,2], fast DVE),
    and 5 macro-tiles of x prefetch cover the remaining latency.
"""

import numpy as np

B, N, DIN, DOUT, DCTX = 16, 16384, 256, 256, 131
NCORES = 8
SPC = B // NCORES           # samples per core
ROWS = SPC * N              # x rows per core
MACRO = 2048                # n-columns of xT per macro-tile
KC = DCTX + 1               # ctx rows incl. the constant-1 row (b_gate)
USE_F32R = False            # kept for test.py compat; fp16 path ignores it


def build_nc(rows=ROWS, use_f32r=USE_F32R):
    import concourse.bass as bass  # noqa: F401
    import concourse.tile as tile
    from concourse import bacc, mybir
    from contextlib import ExitStack

    f32 = mybir.dt.float32
    f16 = mybir.dt.float16
    AF = mybir.ActivationFunctionType
    AX = mybir.AxisListType
    OP = mybir.AluOpType

    n_macro = rows // MACRO
    mps = rows // SPC // MACRO   # macro-tiles per sample
    NQ = MACRO // 512            # 512-col n-chunks per macro
    KB = KC - 128                # ctx rows in the second (short) pack

    nc = bacc.Bacc()
    xT_d = nc.declare_dram_parameter("xT", [256, rows], f16, isOutput=False)
    # cpack: rows = ctx-k (incl const-1 row); cols = [ctx_s | Wg.T | Wb.T | Wkv.T]
    cpA_d = nc.declare_dram_parameter("cpackA", [128, 770], f32, isOutput=False)
    cpB_d = nc.declare_dram_parameter("cpackB", [KB, 770], f32, isOutput=False)
    # pk2: rows = half of d/k; cols = [W_layer (256) | W_layer.T (256) | bl2 (2)]
    pk2a_d = nc.declare_dram_parameter("pk2a", [128, 514], f32, isOutput=False)
    pk2b_d = nc.declare_dram_parameter("pk2b", [128, 514], f32, isOutput=False)
    outT_d = nc.declare_dram_parameter("outT", [256, rows], f16, isOutput=True)

    with tile.TileContext(nc) as tc, ExitStack() as ctx:
        consts = ctx.enter_context(tc.tile_pool(name="consts", bufs=1))
        spool = ctx.enter_context(tc.tile_pool(name="scratch", bufs=2))
        perm = ctx.enter_context(tc.tile_pool(name="persample", bufs=1))
        pps = ctx.enter_context(tc.tile_pool(name="pps", bufs=4, space="PSUM"))
        pout = ctx.enter_context(tc.tile_pool(name="pout", bufs=4, space="PSUM"))
        xin = ctx.enter_context(tc.tile_pool(name="xin", bufs=8))
        osb = ctx.enter_context(tc.tile_pool(name="osb", bufs=3))

        # ---- constants-by-memset + PE warmup (no DMA dependencies) ----
        onesr = consts.tile([1, 128], f32, name="onesr", tag="onesr")
        nc.gpsimd.memset(onesr, 1.0)
        onesc = consts.tile([128, 1], f32, name="onesc", tag="onesc")
        nc.gpsimd.memset(onesc, 1.0)
        wm = consts.tile([128, 512], f16, name="wm", tag="wm")
        nc.gpsimd.memset(wm, 0.0)
        for w in range(4):
            wp_ = pout.tile([128, 512], f32, name=f"warm{w}", tag="op")
            nc.tensor.matmul(wp_, lhsT=wm[:, 0:128], rhs=wm, start=True,
                             stop=True)

        def cload(name, dram_ap, shape, dt=f32):
            t = consts.tile(shape, dt, name=name, tag=name)
            nc.sync.dma_start(t, dram_ap)
            return t

        cpA = cload("cpA", cpA_d[:, :], [128, 770])
        cpB = cload("cpB", cpB_d[:, :], [KB, 770])
        pk2a = cload("pk2a", pk2a_d[:, :], [128, 514])
        pk2b = cload("pk2b", pk2b_d[:, :], [128, 514])
        bl2 = pk2a[:, 512:514]

        # ====== per-sample setup, sequential (s=0's chain gets clean
        # queues; s=1 resolves while the main loop processes s=0) ========
        gateT, weff, bfT = {}, {}, {}
        for s in range(SPC):
            # ---- ctx projections: kv row + gate/bias columns ----
            ckv = pps.tile([1, 256], f32, name=f"ckv{s}", tag="ps")
            nc.tensor.matmul(ckv, lhsT=cpA[:, s:s + 1], rhs=cpA[:, 514:770],
                             start=True, stop=False)
            nc.tensor.matmul(ckv, lhsT=cpB[:, s:s + 1], rhs=cpB[:, 514:770],
                             start=False, stop=True)
            gp = pps.tile([128, 2], f32, name=f"gp{s}", tag="ps")
            bt = pps.tile([128, 2], f32, name=f"bt{s}", tag="ps")
            for h in range(2):
                c0 = 2 + 128 * h
                nc.tensor.matmul(gp[:, h:h + 1], lhsT=cpA[:, c0:c0 + 128],
                                 rhs=cpA[:, s:s + 1], start=True, stop=False)
                nc.tensor.matmul(gp[:, h:h + 1], lhsT=cpB[:, c0:c0 + 128],
                                 rhs=cpB[:, s:s + 1], start=False, stop=True)
                nc.tensor.matmul(bt[:, h:h + 1],
                                 lhsT=cpA[:, c0 + 256:c0 + 384],
                                 rhs=cpA[:, s:s + 1], start=True, stop=False)
                nc.tensor.matmul(bt[:, h:h + 1],
                                 lhsT=cpB[:, c0 + 256:c0 + 384],
                                 rhs=cpB[:, s:s + 1], start=False, stop=True)
            kv = spool.tile([1, 256], f32, name=f"kv{s}", tag="kv")
            nc.vector.tensor_copy(kv, ckv)
            eg = spool.tile([128, 2], f32, name=f"eg{s}", tag="eg")
            nc.scalar.activation(eg, gp, AF.Exp, scale=-1.0)
            btS = spool.tile([128, 2], f32, name=f"btS{s}", tag="btS")
            nc.vector.tensor_copy(btS, bt)
            ga = spool.tile([128, 2], f32, name=f"ga{s}", tag="ga")
            nc.gpsimd.tensor_scalar_add(ga, eg, 1.0)
            gateT[s] = perm.tile([128, 2], f32, name=f"gateT{s}",
                                 tag=f"gateT{s}")
            nc.vector.reciprocal(gateT[s], ga)

            # ---- E = outer(kv, kv); softmax rows (no max-sub needed) ----
            A = {}
            for i in range(2):
                E = pps.tile([128, 256], f32, name=f"E{s}{i}", tag="ps")
                nc.tensor.matmul(E, lhsT=kv[0:1, 128 * i:128 * (i + 1)],
                                 rhs=kv, start=True, stop=True)
                expE = spool.tile([128, 256], f32, name=f"expE{s}{i}",
                                  tag="expE")
                nc.scalar.activation(expE, E, AF.Exp)
                rs = spool.tile([128, 1], f32, name=f"rs{s}{i}", tag="rs")
                nc.vector.reduce_sum(rs, expE, axis=AX.X)
                rc = spool.tile([128, 1], f32, name=f"rc{s}{i}", tag="rc")
                nc.vector.reciprocal(rc, rs)
                A[i] = spool.tile([128, 256], f32, name=f"A{s}{i}",
                                  tag=f"A{s}{i}")
                nc.vector.tensor_scalar_mul(A[i], expE, rc)

            # ---- colsum row -> 1/cs -> broadcast [128,256] ----
            csr = pps.tile([1, 256], f32, name=f"csr{s}", tag="ps")
            nc.tensor.matmul(csr, lhsT=onesc, rhs=A[0], start=True, stop=False)
            nc.tensor.matmul(csr, lhsT=onesc, rhs=A[1], start=False, stop=True)
            rcs = spool.tile([1, 256], f32, name=f"rcs{s}", tag="rcs")
            nc.vector.reciprocal(rcs, csr)
            cb = pps.tile([128, 256], f32, name=f"cb{s}", tag="ps")
            nc.tensor.matmul(cb, lhsT=onesr, rhs=rcs, start=True, stop=True)
            CSi = spool.tile([128, 256], f32, name=f"CSi{s}", tag="CSi")
            nc.vector.tensor_copy(CSi, cb)

            # ---- wp = U @ A ; Q = wp * (1/cs) + U  (fp16) ----
            for j in range(2):
                wpj = pps.tile([128, 256], f32, name=f"wp{s}{j}", tag="ps")
                nc.tensor.matmul(wpj, lhsT=pk2a[:, 128 * j:128 * (j + 1)],
                                 rhs=A[0], start=True, stop=False)
                nc.tensor.matmul(wpj, lhsT=pk2b[:, 128 * j:128 * (j + 1)],
                                 rhs=A[1], start=False, stop=True)
                qm = spool.tile([128, 256], f32, name=f"qm{s}{j}", tag="qm")
                nc.vector.tensor_mul(qm, wpj, CSi)
                weff[(s, j)] = perm.tile([128, 256], f16, name=f"weff{s}{j}",
                                         tag=f"weff{s}{j}")
                U_half = pk2a[:, 256:512] if j == 0 else pk2b[:, 256:512]
                nc.gpsimd.tensor_add(weff[(s, j)], qm, U_half)

            # ---- b_fin columns: g*(bl + (bl@A)/cs) + bias ----
            ct = pps.tile([128, 2], f32, name=f"ct{s}", tag="ps")
            qa = pps.tile([128, 2], f32, name=f"qa{s}", tag="ps")
            for h in range(2):
                hs = slice(128 * h, 128 * (h + 1))
                nc.tensor.matmul(ct[:, h:h + 1], lhsT=A[0][:, hs],
                                 rhs=onesc, start=True, stop=False)
                nc.tensor.matmul(ct[:, h:h + 1], lhsT=A[1][:, hs],
                                 rhs=onesc, start=False, stop=True)
                nc.tensor.matmul(qa[:, h:h + 1], lhsT=A[0][:, hs],
                                 rhs=bl2[:, 0:1], start=True, stop=False)
                nc.tensor.matmul(qa[:, h:h + 1], lhsT=A[1][:, hs],
                                 rhs=bl2[:, 1:2], start=False, stop=True)
            rcT = spool.tile([128, 2], f32, name=f"rcT{s}", tag="rcT")
            nc.vector.reciprocal(rcT, ct)
            f1 = spool.tile([128, 2], f32, name=f"f1{s}", tag="f1")
            nc.vector.tensor_mul(f1, qa, rcT)
            f2 = spool.tile([128, 2], f32, name=f"f2{s}", tag="f2")
            nc.gpsimd.tensor_add(f2, f1, bl2)
            f3 = spool.tile([128, 2], f32, name=f"f3{s}", tag="f3")
            nc.gpsimd.tensor_mul(f3, f2, gateT[s])
            bfT[s] = perm.tile([128, 2], f32, name=f"bfT{s}", tag=f"bfT{s}")
            nc.vector.tensor_add(bfT[s], f3, btS)

        # ================= main loop ====================================
        for t in range(n_macro):
            s = t // mps
            n0 = MACRO * t
            xa = xin.tile([128, MACRO], f16, name="xa", tag="xa")
            nc.sync.dma_start(xa, xT_d[0:128, n0:n0 + MACRO])
            xb = xin.tile([128, MACRO], f16, name="xb", tag="xb")
            nc.sync.dma_start(xb, xT_d[128:256, n0:n0 + MACRO])
            for h in range(2):
                gcol = gateT[s][:, h:h + 1]
                bcol = bfT[s][:, h:h + 1]
                ot = osb.tile([128, MACRO], f16, name=f"ot{h}", tag=f"ot{h}")
                for q in range(NQ):
                    op = pout.tile([128, 512], f32, name="op", tag="op")
                    nc.tensor.matmul(op, lhsT=weff[(s, 0)][:, 128 * h:128 * (h + 1)],
                                     rhs=xa[:, 512 * q:512 * (q + 1)],
                                     start=True, stop=False)
                    nc.tensor.matmul(op, lhsT=weff[(s, 1)][:, 128 * h:128 * (h + 1)],
                                     rhs=xb[:, 512 * q:512 * (q + 1)],
                                     start=False, stop=True)
                    dst = ot[:, 512 * q:512 * (q + 1)]
                    if q % 2 == 0:
                        nc.scalar.activation(dst, op, AF.Identity,
                                             bias=bcol, scale=gcol)
                    else:
                        nc.vector.tensor_scalar(dst, op, gcol, bcol,
                                                op0=OP.mult, op1=OP.add)
                if t >= n_macro - 2:
                    for q in range(NQ):   # finer stores to shorten the drain
                        nc.gpsimd.dma_start(
                            outT_d[128 * h:128 * (h + 1),
                                   n0 + 512 * q:n0 + 512 * (q + 1)],
                            ot[:, 512 * q:512 * (q + 1)])
                else:
                    nc.gpsimd.dma_start(
                        outT_d[128 * h:128 * (h + 1), n0:n0 + MACRO], ot)

    nc.finalize()
    return nc


def prep_host_inputs(ctx, x, W_layer, b_layer, W_bias, W_gate, b_gate, W_kv,
                     rows=ROWS):
    """Build the per-core in_maps (host-side sharding + fp16 relayout)."""
    ctx = np.asarray(ctx, np.float32)
    W_layer = np.asarray(W_layer, np.float32)
    b_layer = np.asarray(b_layer, np.float32)
    W_bias = np.asarray(W_bias, np.float32)
    W_gate = np.asarray(W_gate, np.float32)
    b_gate = np.asarray(b_gate, np.float32)
    W_kv = np.asarray(W_kv, np.float32)
    x16 = np.asarray(x).astype(np.float16)

    wcatT = np.zeros((KC, 768), np.float32)
    wcatT[:DCTX, 0:256] = W_gate.T
    wcatT[:DCTX, 256:512] = W_bias.T
    wcatT[:DCTX, 512:768] = W_kv.T
    wcatT[DCTX, 0:256] = b_gate        # paired with the constant-1 ctx row

    bl2 = b_layer.reshape(2, 128).T            # bl2[p, h] = bl[128h+p]
    pk2a = np.concatenate([W_layer[0:128, :], W_layer.T[0:128, :], bl2],
                          axis=1).astype(np.float32)
    pk2b = np.concatenate([W_layer[128:256, :], W_layer.T[128:256, :], bl2],
                          axis=1).astype(np.float32)
    shared = {"pk2a": np.ascontiguousarray(pk2a),
              "pk2b": np.ascontiguousarray(pk2b)}
    in_maps = []
    for c in range(NCORES):
        ctxc = np.zeros((KC, SPC), np.float32)
        for k in range(SPC):
            ctxc[:DCTX, k] = ctx[SPC * c + k, 0]
            ctxc[DCTX, k] = 1.0
        cpack = np.concatenate([ctxc, wcatT], axis=1)   # [KC, 770]
        xT = np.ascontiguousarray(
            x16[SPC * c:SPC * (c + 1)].reshape(rows, DIN).T)
        in_maps.append({
            "xT": xT,
            "cpackA": np.ascontiguousarray(cpack[0:128]),
            "cpackB": np.ascontiguousarray(cpack[128:KC]),
            **shared,
        })
    return in_maps


def kernel(ctx, x, W_layer, b_layer, W_bias, W_gate, b_gate, W_kv):
    from concourse.bass_utils import run_bass_kernel_spmd

    nc = build_nc(ROWS)
    in_maps = prep_host_inputs(ctx, x, W_layer, b_layer, W_bias, W_gate,
                               b_gate, W_kv)
    res = run_bass_kernel_spmd(nc, in_maps, core_ids=list(range(NCORES)))
    out = np.empty((B, N, DOUT), np.float32)
    for c in range(NCORES):
        outT = res.results[c]["outT"]           # [256, ROWS] fp16
        out[SPC * c:SPC * (c + 1)] = (
            outT.T.reshape(SPC, N, DOUT).astype(np.float32))
    return out
